# revision 15
# baseline (speedup 1.0000x reference)
"""Trainium2 Bass kernel for MiLoLinear: out = x @ (dequant4(W_q) + U@V).T + bias.

Sharding: column-parallel over the 172 dequant groups (gq). Cores 0-3 take 22
groups, cores 4-7 take 21 (+1 zero pad) -> every core computes 1408 output
columns (64 r x 22 gq) of the [512, 11008] output; the host gathers/reorders.

Math per core (all exact rewrites of the reference):
  o = r*172 + gq, r = nib*32 + row, W_q byte = (hi<<4 | lo)
  out[s,o] = sum_c x[s,c]*Q[o,c]*scale[gq,c]            (PE, bf16, dequant on DVE)
           - sum_c x[s,c]*(scale*zero)[gq,c]            (folded: T-rows correction)
           + (x @ V.T) @ U.T + bias                      (folded: y-rows + ones row)
The three corrections ride the same PE accumulation as 55 extra contraction
rows: stationary = [T_T(22); y_T(32); ones(1)], moving = [-indicator; U_T; bias].
"""

import sys

for _p in ("/opt/trn_rl_repo", "/root/.axon_site/_ro/trn_rl_repo"):
    if _p not in sys.path:
        sys.path.append(_p)

import numpy as np
import ml_dtypes

import concourse.bass as bass
import concourse.tile as tile
from concourse import bacc, mybir
from concourse.bass_utils import run_bass_kernel_spmd

OUT_F, IN_F, GROUP = 11008, 4096, 64
G = OUT_F * IN_F // GROUP            # 704512
GQ = G // IN_F                       # 172 groups along out axis
S = 512                              # rows of x
NCORES = 8
GQL = 22                             # padded gq per core
NKT = IN_F // 128                    # 32 contraction tiles
OL = 2 * 32 * GQL                    # 1408 local output columns
NCORR = 55                           # 22 T-rows + 32 y-rows + 1 ones-row
CHUNKS = [(0, 512), (512, 1024), (1024, OL)]

BF16 = ml_dtypes.bfloat16

# gq ownership: cores 0-3 -> 22 groups, cores 4-7 -> 21 (+ pad)
_SIZES = [22, 22, 22, 22, 21, 21, 21, 21]
_STARTS = np.cumsum([0] + _SIZES[:-1]).tolist()


def _core_gqs(k):
    """Global gq indices for core k, padded with -1 to length GQL."""
    gqs = list(range(_STARTS[k], _STARTS[k] + _SIZES[k]))
    return gqs + [-1] * (GQL - len(gqs))


def _build_program():
    nc = bacc.Bacc("TRN2", target_bir_lowering=False, debug=False)
    dt = mybir.dt

    wq_in = nc.declare_dram_parameter("wq", [NKT, 128, OL // 2], dt.uint8, isOutput=False)
    sc_in = nc.declare_dram_parameter("sc", [NKT, 128, OL // 2], dt.bfloat16, isOutput=False)
    xt_in = nc.declare_dram_parameter("xt", [NKT, 128, S], dt.bfloat16, isOutput=False)
    zv_in = nc.declare_dram_parameter("zv", [NKT, 128, NCORR - 1], dt.bfloat16, isOutput=False)
    cr_in = nc.declare_dram_parameter("cr", [NCORR, OL], dt.bfloat16, isOutput=False)
    out_d = nc.declare_dram_parameter("out", [S // 128, 128, OL], dt.float32, isOutput=True)

    NST = S // 128
    with tile.TileContext(nc) as tc:
        with (
            tc.tile_pool(name="const", bufs=1) as cpool,
            tc.tile_pool(name="wq", bufs=2) as wqp,
            tc.tile_pool(name="sc", bufs=2) as scp,
            tc.tile_pool(name="nib", bufs=3) as nibp,
            tc.tile_pool(name="out", bufs=3) as outp,
            tc.tile_pool(name="ps", bufs=4, space="PSUM") as psp,
        ):
            # ---- dequant input DMAs first (scalar/HWDGE queue, t order) ----
            H = OL // 2
            wq_t, sc_t = [], []
            for t in range(NKT):
                wq = wqp.tile([128, H], dt.uint8, tag="wq")
                nc.scalar.dma_start(wq[:], wq_in[t])
                sc = scp.tile([128, H], dt.bfloat16, tag="sc")
                nc.scalar.dma_start(sc[:], sc_in[t])
                wq_t.append(wq)
                sc_t.append(sc)

            # ---- resident constants (sync queue): zv first, then xt ----
            xt = cpool.tile([128, NKT * S], dt.bfloat16)
            zv = cpool.tile([128, NKT * (NCORR - 1)], dt.bfloat16)
            nc.sync.dma_start(
                zv[:].rearrange("p (q s) -> p q s", q=NKT),
                zv_in[:].rearrange("q p s -> p q s"),
            )
            for i in range(8):
                t = i * 4
                nc.sync.dma_start(
                    xt[:, t * S:(t + 4) * S].rearrange("p (q s) -> p q s", q=4),
                    xt_in[t:t + 4].rearrange("q p s -> p q s"),
                )
            cr = cpool.tile([NCORR, OL], dt.bfloat16)
            nc.sync.dma_start(cr[:], cr_in[:])
            wbf = cpool.tile([128, NKT * OL], dt.bfloat16)

            # ---- phase A: correction stationary rows [T_T; y_T; ones] ----
            corr = cpool.tile([NCORR, S], dt.bfloat16)
            nc.vector.memset(corr[:], 1.0)          # row 54 stays the ones-row
            pa = psp.tile([NCORR - 1, S], dt.float32, tag="ps")
            # warm the PE (HAM clock gate) while input DMAs stream
            for _ in range(8):
                nc.tensor.matmul(pa[:], corr[:, 0:NCORR - 1], corr[:],
                                 start=True, stop=True)
            for t in range(NKT):
                nc.tensor.matmul(
                    pa[:], zv[:, t * (NCORR - 1):(t + 1) * (NCORR - 1)],
                    xt[:, t * S:(t + 1) * S],
                    start=(t == 0), stop=(t == NKT - 1),
                )
            # ---- dequant: W'[c, j] = nibble(Wq) * scale, bf16, resident ----
            # hi path (shift+mask + mult) on DVE; lo mult on GpSimd (parallel,
            # tensor_tensor never grabs the shared port pair).
            for t in range(NKT):
                wq, sc = wq_t[t], sc_t[t]
                hi4 = nibp.tile([128, H // 4], dt.uint32, tag="hi4")
                lo4 = nibp.tile([128, H // 4], dt.uint32, tag="lo4")
                nc.vector.tensor_scalar(
                    hi4[:], wq[:].bitcast(dt.uint32), 4, 0x0F0F0F0F,
                    op0=mybir.AluOpType.logical_shift_right,
                    op1=mybir.AluOpType.bitwise_and)
                nc.vector.tensor_scalar(
                    lo4[:], wq[:].bitcast(dt.uint32), 0x0F0F0F0F, None,
                    op0=mybir.AluOpType.bitwise_and)
                nc.vector.tensor_tensor(
                    wbf[:, t * OL:t * OL + H],
                    hi4[:].bitcast(dt.uint8), sc[:], op=mybir.AluOpType.mult)
                nc.vector.tensor_tensor(
                    wbf[:, t * OL + H:(t + 1) * OL],
                    lo4[:].bitcast(dt.uint8), sc[:], op=mybir.AluOpType.mult)
                if t == 2:
                    # drain phase-A psum -> corr rows (frees psum slot 0)
                    nc.vector.tensor_copy(corr[0:NCORR - 1, :], pa[:])

            # ---- pass A: output cols [0, 1024), t-outer so dequant streams ----
            CA, CB = 1024, OL - 1024
            psa = [psp.tile([128, CA], dt.float32, tag="ps", name=f"psa{i}") for i in range(NST)]
            for t in range(NKT):
                for st in range(NST):
                    lhs = xt[:, t * S + st * 128: t * S + (st + 1) * 128]
                    for a, b in [(0, 512), (512, 1024)]:
                        nc.tensor.matmul(
                            psa[st][:, a:b], lhs, wbf[:, t * OL + a:t * OL + b],
                            start=(t == 0), stop=False)
            for st in range(NST):
                clhs = corr[:, st * 128:(st + 1) * 128]
                for a, b in [(0, 512), (512, 1024)]:
                    nc.tensor.matmul(psa[st][:, a:b], clhs, cr[:, a:b],
                                     start=False, stop=True)
                ot = outp.tile([128, CA], dt.float32, tag="out")
                nc.scalar.copy(ot[:], psa[st][:])
                nc.sync.dma_start(out_d[st][:, 0:CA], ot[:])

            # ---- pass B: output cols [1024, 1408) ----
            psb = [psp.tile([128, CB], dt.float32, tag="ps", name=f"psb{i}") for i in range(NST)]
            for t in range(NKT):
                for st in range(NST):
                    lhs = xt[:, t * S + st * 128: t * S + (st + 1) * 128]
                    nc.tensor.matmul(
                        psb[st][:], lhs, wbf[:, t * OL + CA:(t + 1) * OL],
                        start=(t == 0), stop=False)
            for st in range(NST):
                clhs = corr[:, st * 128:(st + 1) * 128]
                nc.tensor.matmul(psb[st][:], clhs, cr[:, CA:OL],
                                 start=False, stop=True)
                ot = outp.tile([128, CB], dt.float32, tag="out")
                nc.scalar.copy(ot[:], psb[st][:])
                nc.sync.dma_start(out_d[st][:, CA:OL], ot[:])

    nc.compile()
    return nc


def _prep_inputs(x, W_q, scale, zero, U, V, bias):
    """Build the 8 per-core input maps (all host-side numpy)."""
    Wq_u8 = W_q.astype(np.uint8).reshape(32, GQ, IN_F)
    scale_g = scale.reshape(GQ, IN_F).astype(np.float32)
    zero_g = zero.reshape(GQ, IN_F).astype(np.float32)
    sz_g = scale_g * zero_g

    xt = np.ascontiguousarray(x.T).astype(BF16).reshape(NKT, 128, S)

    in_maps = []
    o_maps = []
    for k in range(NCORES):
        gqs = _core_gqs(k)
        valid = np.array([g >= 0 for g in gqs])
        gq_idx = np.array([g if g >= 0 else 0 for g in gqs])

        # packed bytes: [row32, gq22, c4096] -> [c, row, gq] -> [32, 128, 704]
        A = Wq_u8[:, gq_idx, :].copy()
        A[:, ~valid, :] = 0
        wq_dev = np.ascontiguousarray(A.transpose(2, 0, 1)).reshape(NKT, 128, OL // 2)

        # scale replicated over row: [c, row, gq] bf16
        Sg = scale_g[gq_idx].copy()
        Sg[~valid] = 0.0
        sc_dev = np.ascontiguousarray(
            np.broadcast_to(Sg.T[:, None, :], (IN_F, 32, GQL))
        ).astype(BF16).reshape(NKT, 128, OL // 2)

        # zv: [c, 22 sz-rows + 32 V-rows]
        Zg = sz_g[gq_idx].copy()
        Zg[~valid] = 0.0
        zv_dev = np.ascontiguousarray(
            np.concatenate([Zg.T, V.T.astype(np.float32)], axis=1)
        ).astype(BF16).reshape(NKT, 128, NCORR - 1)

        # local output column map: j = nib*704 + row*22 + gq -> global o
        nib = np.arange(OL) // (OL // 2)
        row = (np.arange(OL) % (OL // 2)) // GQL
        gql = np.arange(OL) % GQL
        r = nib * 32 + row
        gq_glob = np.array(gqs)[gql]
        o_map = np.where(gq_glob >= 0, r * GQ + gq_glob, -1)
        o_maps.append(o_map)

        # correction moving rows: [-indicator(22); U_T(32); bias(1)]
        cr_dev = np.zeros((NCORR, OL), dtype=np.float32)
        ind = gql[None, :] == np.arange(GQL)[:, None]      # [22, 1408]
        cr_dev[:GQL] = np.where(ind, -1.0, 0.0)
        ok = o_map >= 0
        cr_dev[:GQL, ~ok] = 0.0
        cr_dev[GQL:GQL + 32, ok] = U[o_map[ok]].astype(np.float32).T
        cr_dev[NCORR - 1, ok] = bias[o_map[ok]].astype(np.float32)
        cr_dev = cr_dev.astype(BF16)

        in_maps.append({
            "wq": wq_dev, "sc": sc_dev, "xt": xt, "zv": zv_dev, "cr": cr_dev,
        })
    return in_maps, o_maps


_CACHE = {}


def kernel(x, W_q, scale, zero, U, V, bias):
    x = np.asarray(x)
    W_q = np.asarray(W_q)
    scale = np.asarray(scale)
    zero = np.asarray(zero)
    U = np.asarray(U)
    V = np.asarray(V)
    bias = np.asarray(bias)

    if "nc" not in _CACHE:
        _CACHE["nc"] = _build_program()
    nc = _CACHE["nc"]

    in_maps, o_maps = _prep_inputs(x, W_q, scale, zero, U, V, bias)
    res = run_bass_kernel_spmd(nc, in_maps, list(range(NCORES)))

    out = np.zeros((S, OUT_F), dtype=np.float32)
    for k in range(NCORES):
        oc = res.results[k]["out"].reshape(S, OL)
        ok = o_maps[k] >= 0
        out[:, o_maps[k][ok]] = oc[:, ok]
    return out


# revision 16
# speedup vs baseline: 1.0516x; 1.0516x over previous
"""Trainium2 Bass kernel for MiLoLinear: out = x @ (dequant4(W_q) + U@V).T + bias.

Sharding: column-parallel over the 172 dequant groups (gq). Cores 0-3 take 22
groups, cores 4-7 take 21 (+1 zero pad) -> every core computes 1408 output
columns (64 r x 22 gq) of the [512, 11008] output; the host gathers/reorders.

Math per core (all exact rewrites of the reference):
  o = r*172 + gq, r = nib*32 + row, W_q byte = (hi<<4 | lo)
  out[s,o] = sum_c x[s,c]*Q[o,c]*scale[gq,c]            (PE, bf16, dequant on DVE)
           - sum_c x[s,c]*(scale*zero)[gq,c]            (folded: T-rows correction)
           + (x @ V.T) @ U.T + bias                      (folded: y-rows + ones row)
The three corrections ride the same PE accumulation as 55 extra contraction
rows: stationary = [T_T(22); y_T(32); ones(1)], moving = [-indicator; U_T; bias].
"""

import sys

for _p in ("/opt/trn_rl_repo", "/root/.axon_site/_ro/trn_rl_repo"):
    if _p not in sys.path:
        sys.path.append(_p)

import numpy as np
import ml_dtypes

import concourse.bass as bass
import concourse.tile as tile
from concourse import bacc, mybir
from concourse.bass_utils import run_bass_kernel_spmd

OUT_F, IN_F, GROUP = 11008, 4096, 64
G = OUT_F * IN_F // GROUP            # 704512
GQ = G // IN_F                       # 172 groups along out axis
S = 512                              # rows of x
NCORES = 8
GQL = 22                             # padded gq per core
NKT = IN_F // 128                    # 32 contraction tiles
OL = 2 * 32 * GQL                    # 1408 local output columns
NCORR = 55                           # 22 T-rows + 32 y-rows + 1 ones-row
CHUNKS = [(0, 512), (512, 1024), (1024, OL)]

BF16 = ml_dtypes.bfloat16

# gq ownership: cores 0-3 -> 22 groups, cores 4-7 -> 21 (+ pad)
_SIZES = [22, 22, 22, 22, 21, 21, 21, 21]
_STARTS = np.cumsum([0] + _SIZES[:-1]).tolist()


def _core_gqs(k):
    """Global gq indices for core k, padded with -1 to length GQL."""
    gqs = list(range(_STARTS[k], _STARTS[k] + _SIZES[k]))
    return gqs + [-1] * (GQL - len(gqs))


def _build_program():
    nc = bacc.Bacc("TRN2", target_bir_lowering=False, debug=False)
    dt = mybir.dt

    wq_in = nc.declare_dram_parameter("wq", [NKT, 128, OL // 2], dt.uint8, isOutput=False)
    sc_in = nc.declare_dram_parameter("sc", [NKT, 128, OL // 2], dt.bfloat16, isOutput=False)
    xt_in = nc.declare_dram_parameter("xt", [NKT, 128, S], dt.bfloat16, isOutput=False)
    zv_in = nc.declare_dram_parameter("zv", [NKT, 128, NCORR - 1], dt.bfloat16, isOutput=False)
    cr_in = nc.declare_dram_parameter("cr", [NCORR, OL], dt.bfloat16, isOutput=False)
    out_d = nc.declare_dram_parameter("out", [S // 128, 128, OL], dt.float32, isOutput=True)

    NST = S // 128
    with tile.TileContext(nc) as tc:
        with (
            tc.tile_pool(name="const", bufs=1) as cpool,
            tc.tile_pool(name="wq", bufs=4) as wqp,
            tc.tile_pool(name="sc", bufs=4) as scp,
            tc.tile_pool(name="nib", bufs=3) as nibp,
            tc.tile_pool(name="out", bufs=3) as outp,
            tc.tile_pool(name="ps", bufs=4, space="PSUM") as psp,
        ):
            # ---- dequant input DMAs first (scalar/HWDGE queue, t order) ----
            H = OL // 2
            wq_t, sc_t = [], []
            for t in range(NKT):
                wq = wqp.tile([128, H], dt.uint8, tag="wq")
                nc.scalar.dma_start(wq[:], wq_in[t])
                sc = scp.tile([128, H], dt.bfloat16, tag="sc")
                nc.scalar.dma_start(sc[:], sc_in[t])
                wq_t.append(wq)
                sc_t.append(sc)

            # ---- resident constants (sync queue): zv first, then xt ----
            xt = cpool.tile([128, NKT * S], dt.bfloat16)
            zv = cpool.tile([128, NKT * (NCORR - 1)], dt.bfloat16)
            nc.sync.dma_start(
                zv[:].rearrange("p (q s) -> p q s", q=NKT),
                zv_in[:].rearrange("q p s -> p q s"),
            )
            for i in range(8):
                t = i * 4
                nc.sync.dma_start(
                    xt[:, t * S:(t + 4) * S].rearrange("p (q s) -> p q s", q=4),
                    xt_in[t:t + 4].rearrange("q p s -> p q s"),
                )
            cr = cpool.tile([NCORR, OL], dt.bfloat16)
            nc.sync.dma_start(cr[:], cr_in[:])
            wbf = cpool.tile([128, NKT * OL], dt.bfloat16)

            # ---- phase A: correction stationary rows [T_T; y_T; ones] ----
            corr = cpool.tile([NCORR, S], dt.bfloat16)
            nc.vector.memset(corr[:], 1.0)          # row 54 stays the ones-row
            pa = psp.tile([NCORR - 1, S], dt.float32, tag="ps")
            # warm the PE (HAM clock gate) while input DMAs stream
            for _ in range(8):
                nc.tensor.matmul(pa[:], corr[:, 0:NCORR - 1], corr[:],
                                 start=True, stop=True)
            for t in range(NKT):
                nc.tensor.matmul(
                    pa[:], zv[:, t * (NCORR - 1):(t + 1) * (NCORR - 1)],
                    xt[:, t * S:(t + 1) * S],
                    start=(t == 0), stop=(t == NKT - 1),
                )
            # ---- dequant: W'[c, j] = nibble(Wq) * scale, bf16, resident ----
            # hi path (shift+mask + mult) on DVE; lo mult on GpSimd (parallel,
            # tensor_tensor never grabs the shared port pair).
            for t in range(NKT):
                wq, sc = wq_t[t], sc_t[t]
                hi4 = nibp.tile([128, H // 4], dt.uint32, tag="hi4")
                lo4 = nibp.tile([128, H // 4], dt.uint32, tag="lo4")
                nc.vector.tensor_scalar(
                    hi4[:], wq[:].bitcast(dt.uint32), 4, 0x0F0F0F0F,
                    op0=mybir.AluOpType.logical_shift_right,
                    op1=mybir.AluOpType.bitwise_and)
                nc.vector.tensor_scalar(
                    lo4[:], wq[:].bitcast(dt.uint32), 0x0F0F0F0F, None,
                    op0=mybir.AluOpType.bitwise_and)
                nc.vector.tensor_tensor(
                    wbf[:, t * OL:t * OL + H],
                    hi4[:].bitcast(dt.uint8), sc[:], op=mybir.AluOpType.mult)
                nc.vector.tensor_tensor(
                    wbf[:, t * OL + H:(t + 1) * OL],
                    lo4[:].bitcast(dt.uint8), sc[:], op=mybir.AluOpType.mult)
                if t == 2:
                    # drain phase-A psum -> corr rows (frees psum slot 0)
                    nc.vector.tensor_copy(corr[0:NCORR - 1, :], pa[:])

            # ---- pass A: output cols [0, 1024), t-outer so dequant streams ----
            CA, CB = 1024, OL - 1024
            psa = [psp.tile([128, CA], dt.float32, tag="ps", name=f"psa{i}") for i in range(NST)]
            for t in range(NKT):
                for st in range(NST):
                    lhs = xt[:, t * S + st * 128: t * S + (st + 1) * 128]
                    for a, b in [(0, 512), (512, 1024)]:
                        nc.tensor.matmul(
                            psa[st][:, a:b], lhs, wbf[:, t * OL + a:t * OL + b],
                            start=(t == 0), stop=False)
            for st in range(NST):
                clhs = corr[:, st * 128:(st + 1) * 128]
                for a, b in [(0, 512), (512, 1024)]:
                    nc.tensor.matmul(psa[st][:, a:b], clhs, cr[:, a:b],
                                     start=False, stop=True)
                ot = outp.tile([128, CA], dt.float32, tag="out")
                nc.scalar.copy(ot[:], psa[st][:])
                nc.sync.dma_start(out_d[st][:, 0:CA], ot[:])

            # ---- pass B: output cols [1024, 1408) ----
            psb = [psp.tile([128, CB], dt.float32, tag="ps", name=f"psb{i}") for i in range(NST)]
            for t in range(NKT):
                for st in range(NST):
                    lhs = xt[:, t * S + st * 128: t * S + (st + 1) * 128]
                    nc.tensor.matmul(
                        psb[st][:], lhs, wbf[:, t * OL + CA:(t + 1) * OL],
                        start=(t == 0), stop=False)
            for st in range(NST):
                clhs = corr[:, st * 128:(st + 1) * 128]
                nc.tensor.matmul(psb[st][:], clhs, cr[:, CA:OL],
                                 start=False, stop=True)
                ot = outp.tile([128, CB], dt.float32, tag="out")
                nc.scalar.copy(ot[:], psb[st][:])
                nc.sync.dma_start(out_d[st][:, CA:OL], ot[:])

    nc.compile()
    return nc


def _prep_inputs(x, W_q, scale, zero, U, V, bias):
    """Build the 8 per-core input maps (all host-side numpy)."""
    Wq_u8 = W_q.astype(np.uint8).reshape(32, GQ, IN_F)
    scale_g = scale.reshape(GQ, IN_F).astype(np.float32)
    zero_g = zero.reshape(GQ, IN_F).astype(np.float32)
    sz_g = scale_g * zero_g

    xt = np.ascontiguousarray(x.T).astype(BF16).reshape(NKT, 128, S)

    in_maps = []
    o_maps = []
    for k in range(NCORES):
        gqs = _core_gqs(k)
        valid = np.array([g >= 0 for g in gqs])
        gq_idx = np.array([g if g >= 0 else 0 for g in gqs])

        # packed bytes: [row32, gq22, c4096] -> [c, row, gq] -> [32, 128, 704]
        A = Wq_u8[:, gq_idx, :].copy()
        A[:, ~valid, :] = 0
        wq_dev = np.ascontiguousarray(A.transpose(2, 0, 1)).reshape(NKT, 128, OL // 2)

        # scale replicated over row: [c, row, gq] bf16
        Sg = scale_g[gq_idx].copy()
        Sg[~valid] = 0.0
        sc_dev = np.ascontiguousarray(
            np.broadcast_to(Sg.T[:, None, :], (IN_F, 32, GQL))
        ).astype(BF16).reshape(NKT, 128, OL // 2)

        # zv: [c, 22 sz-rows + 32 V-rows]
        Zg = sz_g[gq_idx].copy()
        Zg[~valid] = 0.0
        zv_dev = np.ascontiguousarray(
            np.concatenate([Zg.T, V.T.astype(np.float32)], axis=1)
        ).astype(BF16).reshape(NKT, 128, NCORR - 1)

        # local output column map: j = nib*704 + row*22 + gq -> global o
        nib = np.arange(OL) // (OL // 2)
        row = (np.arange(OL) % (OL // 2)) // GQL
        gql = np.arange(OL) % GQL
        r = nib * 32 + row
        gq_glob = np.array(gqs)[gql]
        o_map = np.where(gq_glob >= 0, r * GQ + gq_glob, -1)
        o_maps.append(o_map)

        # correction moving rows: [-indicator(22); U_T(32); bias(1)]
        cr_dev = np.zeros((NCORR, OL), dtype=np.float32)
        ind = gql[None, :] == np.arange(GQL)[:, None]      # [22, 1408]
        cr_dev[:GQL] = np.where(ind, -1.0, 0.0)
        ok = o_map >= 0
        cr_dev[:GQL, ~ok] = 0.0
        cr_dev[GQL:GQL + 32, ok] = U[o_map[ok]].astype(np.float32).T
        cr_dev[NCORR - 1, ok] = bias[o_map[ok]].astype(np.float32)
        cr_dev = cr_dev.astype(BF16)

        in_maps.append({
            "wq": wq_dev, "sc": sc_dev, "xt": xt, "zv": zv_dev, "cr": cr_dev,
        })
    return in_maps, o_maps


_CACHE = {}


def kernel(x, W_q, scale, zero, U, V, bias):
    x = np.asarray(x)
    W_q = np.asarray(W_q)
    scale = np.asarray(scale)
    zero = np.asarray(zero)
    U = np.asarray(U)
    V = np.asarray(V)
    bias = np.asarray(bias)

    if "nc" not in _CACHE:
        _CACHE["nc"] = _build_program()
    nc = _CACHE["nc"]

    in_maps, o_maps = _prep_inputs(x, W_q, scale, zero, U, V, bias)
    res = run_bass_kernel_spmd(nc, in_maps, list(range(NCORES)))

    out = np.zeros((S, OUT_F), dtype=np.float32)
    for k in range(NCORES):
        oc = res.results[k]["out"].reshape(S, OL)
        ok = o_maps[k] >= 0
        out[:, o_maps[k][ok]] = oc[:, ok]
    return out
